# revision 15
# baseline (speedup 1.0000x reference)
"""GCN block (GCNConv + ReLU + GraphNorm) on 8 Trainium2 NeuronCores.

Strategy (SPMD, one program for all 8 cores; per-core variation lives in data):
  - Shard graphs 8-per-core; each core owns its graphs' nodes (contiguous,
    since batch_index is sorted) and the edges whose dst lands there.
  - Algebraic rewrite: segment_sum((x@W)[src]*attr) == segment_sum(x[src]*attr) @ W,
    so cores gather raw x rows (fp16) instead of computing full h = x@W.
  - Host balancer assigns each graph's nodes to NB blocks of <=128 slots such
    that every (block, src-parity-chunk) edge group has exactly CAP[c] stream
    slots (real edges + duplicate-idx/attr=0 pads) -> the device schedule
    (which 128-edge tile feeds which block PSUM accumulator) is fully static
    and identical on every core.
  - Device inner loop per 128-edge tile visit:
      S = (iota == drel) * attr   (one fused DVE tensor_scalar, fp16)
      psum[block] += Xg_tile.T @ S (PE matmul accumulate, out = [din, nodes])
  - Then per graph: drain PSUM -> SBUF, xaggT @ W via PE (fp32), ACT relu+bias
    (+row sums), GraphNorm from raw moments (hole-corrected), final normalize,
    DMA out. Host scatters slots back to original node order.
"""
import sys
sys.path.insert(0, '/opt/trn_rl_repo')

import numpy as np

N_CORES = 8
D = 128
P = 128           # partitions / tile lanes
NCHUNK = 4        # src parity chunks (int16 gather index reach)
EPS = 1e-5
MSG_DT = np.float16

LAST_RESULT = None  # BassKernelResults of the most recent run (for profiling)

_PLAN_CACHE = {}


# --------------------------------------------------------------------------
# Host-side plan construction
# --------------------------------------------------------------------------

class Plan:
    pass


def _build_plan(x, edge_index, edge_attr, batch_index):
    N = x.shape[0]
    E = edge_index.shape[1]
    G = int(batch_index[-1]) + 1 if len(batch_index) else 1
    G = max(G, 1)
    # graphs per core (sharding hint: shard by graph id)
    assert G % N_CORES == 0, G
    GPC = G // N_CORES

    src = edge_index[0].astype(np.int64)
    dst = edge_index[1].astype(np.int64)
    attr = edge_attr.astype(np.float32)

    gph_of_node = batch_index.astype(np.int64)          # sorted
    gph_e = gph_of_node[dst]                            # graph of each edge
    chunk_e = (src & 3).astype(np.int64)                # src parity chunk

    # node ranges per graph
    gstart = np.searchsorted(gph_of_node, np.arange(G), side='left')
    gend = np.searchsorted(gph_of_node, np.arange(G), side='right')
    n_g = gend - gstart                                 # nodes per graph

    # per (graph, chunk) edge counts
    egc = np.zeros((G, NCHUNK), np.int64)
    np.add.at(egc, (gph_e, chunk_e), 1)

    # per-node per-chunk in-degree
    deg = np.zeros((N, NCHUNK), np.int32)
    np.add.at(deg, (dst, chunk_e), 1)

    # ---- static dims (uniform across cores) ----
    NB = int(np.ceil(n_g.max() / P)) + 1                # blocks per graph
    L = NB * P                                          # slots per graph
    cap = np.ceil(egc.max(axis=0) / NB).astype(np.int64) + 16   # edges/block/chunk

    # ---- balancer: assign nodes of each graph to blocks ----
    # constraints per block: <=128 nodes, <=cap[c] edges for each chunk c
    block_of_node = np.full(N, -1, np.int64)
    slot_of_node = np.full(N, -1, np.int64)             # slot within graph [0, L)
    for attempt in range(6):
        ok = True
        for g in range(G):
            nodes = np.arange(gstart[g], gend[g])
            dg = deg[nodes]                             # [n_g, 4]
            order = np.argsort(-dg.sum(axis=1), kind='stable')
            rem = np.tile(cap, (NB, 1)).astype(np.int64)
            cnt = np.zeros(NB, np.int64)
            blk = np.empty(len(nodes), np.int64)
            for i in order:
                d4 = dg[i]
                placed = False
                for b in range(NB):
                    if cnt[b] < P and np.all(rem[b] >= d4):
                        rem[b] -= d4
                        cnt[b] += 1
                        blk[i] = b
                        placed = True
                        break
                if not placed:
                    ok = False
                    break
            if not ok:
                break
            block_of_node[nodes] = blk
            # slots: nodes packed in block assignment order
            o2 = np.lexsort((np.arange(len(nodes)), blk))
            s = np.empty(len(nodes), np.int64)
            pos = 0
            prev_b = -1
            for j in o2:
                b = blk[j]
                if b != prev_b:
                    pos = b * P
                    prev_b = b
                s[j] = pos
                pos += 1
            slot_of_node[nodes] = s
        if ok:
            break
        cap = cap + 32                                  # more slack, retry
    assert ok, "balancer failed"

    ST = int(np.ceil(NB * cap.max() / P))               # tiles per segment (uniform)
    SEG_T = [ST] * NCHUNK

    # ---- visit schedule (static, identical for all cores/graphs) ----
    # per (c, b): tiles [floor(cap*b/128), ceil(cap*(b+1)/128)) of the segment
    # BLOCK-major order: a block's accumulation group must not interleave with
    # other blocks in the same PSUM bank (start=True clears whole-bank
    # has_written bits), so all 4 chunks of block b run before block b+1.
    visits = []      # list of (g, c, b, t, first, last) in device order
    win = {}
    for c in range(NCHUNK):
        for b in range(NB):
            lo = (cap[c] * b) // P
            hi = -(-(cap[c] * (b + 1)) // P)
            hi = min(hi, SEG_T[c])
            win[(c, b)] = (int(lo), int(hi))
    for g in range(GPC):
        for b in range(NB):
            for c in range(NCHUNK):
                lo, hi = win[(c, b)]
                for t in range(lo, hi):
                    first = (c == 0 and t == lo)
                    last = (c == NCHUNK - 1 and t == hi - 1)
                    visits.append((g, c, b, t, first, last))

    # ---- per-core data arrays ----
    IC = sum(SEG_T) * 8 * GPC                           # idx cols (int16, wrapped)
    VTOT = len(visits)
    plan = Plan()
    plan.N, plan.E, plan.G, plan.GPC = N, E, G, GPC
    plan.NB, plan.L, plan.cap, plan.SEG_T = NB, L, cap, SEG_T
    plan.visits, plan.win = visits, win
    plan.IC, plan.VTOT = IC, VTOT
    plan.gstart, plan.gend, plan.n_g = gstart, gend, n_g
    plan.slot_of_node, plan.block_of_node = slot_of_node, block_of_node

    # order edges once: by (graph, chunk, block, slot)
    blk_e = block_of_node[dst]
    slot_e = slot_of_node[dst]
    eorder = np.lexsort((slot_e, blk_e, chunk_e, gph_e))
    plan.eorder = eorder
    plan.src_s = src[eorder]
    plan.attr_s = attr[eorder]
    plan.slot_s = slot_e[eorder]
    plan.gph_s = gph_e[eorder]
    plan.chunk_s = chunk_e[eorder]
    plan.blk_s = blk_e[eorder]

    # group boundaries: count per (g, c, b)
    cnt_gcb = np.zeros((G, NCHUNK, NB), np.int64)
    np.add.at(cnt_gcb, (gph_e, chunk_e, blk_e), 1)
    assert (cnt_gcb <= cap[None, :, None]).all()
    plan.cnt_gcb = cnt_gcb
    return plan


def _wrap_idxs(i1d):
    """[n] int16 (n % 16 == 0) -> [128, n//16] wrapped + replicated layout."""
    a = i1d.reshape(-1, 16).T                           # [16, cols]
    return np.tile(a, (8, 1)).copy()                    # [128, cols]


def _build_core_arrays(plan, core):
    """Build idx/drel/attr streams for one core. Returns (idx16, drel, attr)."""
    GPC, NB, NCH = plan.GPC, plan.NB, NCHUNK
    cap, SEG_T = plan.cap, plan.SEG_T
    P_ = P

    idx_cols = []
    drel_v = np.zeros((P_, plan.VTOT), np.float32)
    attr_v = np.zeros((P_, plan.VTOT), np.float32)

    # per (g, c): build the padded edge stream
    stream_cache = {}
    for g in range(GPC):
        gg = core * GPC + g
        for c in range(NCH):
            n_stream = NB * cap[c]
            idx = np.zeros(SEG_T[c] * P_, np.int16)
            slot_rel = np.full(SEG_T[c] * P_, -1.0, np.float32)   # graph-rel slot
            a_st = np.zeros(SEG_T[c] * P_, np.float32)
            # locate this (g, c) run in the sorted edge arrays
            base = np.searchsorted(plan.gph_s * NCH + plan.chunk_s, gg * NCH + c,
                                   side='left')
            pos = 0
            ptr = base
            for b in range(plan.NB):
                k = plan.cnt_gcb[gg, c, b]
                if k:
                    sl = slice(ptr, ptr + k)
                    idx[pos:pos + k] = plan.src_s[sl] >> 2      # src // 4
                    slot_rel[pos:pos + k] = (plan.slot_s[sl] - 0).astype(np.float32)
                    a_st[pos:pos + k] = plan.attr_s[sl]
                    ptr += k
                npad = cap[c] - k
                if npad:
                    # duplicate a valid idx (or 0) with attr 0
                    fill = idx[pos + k - 1] if k else np.int16(0)
                    idx[pos + k: pos + cap[c]] = fill
                pos += cap[c]
            # trailing tile pad: idx -1 (trimmed by HW, no DMA traffic)
            idx[n_stream:] = -1
            idx_cols.append(_wrap_idxs(idx))
            stream_cache[(g, c)] = (slot_rel, a_st, n_stream)

    # visits: fill drel/attr columns
    for v, (g, c, b, t, _f, _l) in enumerate(plan.visits):
        slot_rel, a_st, _ns = stream_cache[(g, c)]
        sl = slice(t * P_, (t + 1) * P_)
        drel_v[:, v] = slot_rel[sl] - b * P_
        attr_v[:, v] = a_st[sl]

    idx16 = np.concatenate(idx_cols, axis=1)            # [128, IC]
    assert idx16.shape[1] == plan.IC
    return idx16, drel_v, attr_v


# --------------------------------------------------------------------------
# Device program
# --------------------------------------------------------------------------

def _build_program(plan):
    import concourse.bacc as bacc
    import concourse.tile as tile
    from concourse import mybir

    GPC, NB = plan.GPC, plan.NB
    L = plan.L
    SEG_T = plan.SEG_T
    HTOT = GPC * L                                      # h slots per core

    nc = bacc.Bacc(trn_type="TRN2")
    dt = mybir.dt
    x16_d = nc.dram_tensor("x16", [plan.N // 4, 4 * D], dt.float16,
                           kind="ExternalInput")        # x rows, parity-chunked
    idx_d = nc.dram_tensor("idx", [P, plan.IC], dt.int16, kind="ExternalInput")
    drel_d = nc.dram_tensor("drel", [P, plan.VTOT], dt.float32, kind="ExternalInput")
    attr_d = nc.dram_tensor("attr", [P, plan.VTOT], dt.float32, kind="ExternalInput")
    iota_d = nc.dram_tensor("iota", [P, P], dt.float16, kind="ExternalInput")
    w_d = nc.dram_tensor("w", [D, D], dt.float32, kind="ExternalInput")
    bvec_d = nc.dram_tensor("bvec", [P, 1], dt.float32, kind="ExternalInput")
    gnw_d = nc.dram_tensor("gnw", [P, 1], dt.float32, kind="ExternalInput")
    gnb_d = nc.dram_tensor("gnb", [P, 1], dt.float32, kind="ExternalInput")
    gms_d = nc.dram_tensor("gms", [P, 1], dt.float32, kind="ExternalInput")
    ginv_d = nc.dram_tensor("ginv", [P, GPC], dt.float32, kind="ExternalInput")
    ghole_d = nc.dram_tensor("ghole", [P, GPC], dt.float32, kind="ExternalInput")
    out_d = nc.dram_tensor("out", [P, HTOT], dt.float32, kind="ExternalOutput")

    AF = mybir.ActivationFunctionType
    OP = mybir.AluOpType

    with tile.TileContext(nc) as tc:
        with tc.tile_pool(name="const", bufs=1) as cpool, \
             tc.tile_pool(name="meta", bufs=1) as mpool, \
             tc.tile_pool(name="big", bufs=1) as bigpool, \
             tc.tile_pool(name="gb", bufs=12) as gpool, \
             tc.tile_pool(name="s", bufs=4) as spool, \
             tc.tile_pool(name="sc", bufs=2) as scpool, \
             tc.tile_pool(name="agg", bufs=6, space="PSUM") as apool, \
             tc.tile_pool(name="wps", bufs=2, space="PSUM") as wpool:

            iota_t = cpool.tile([P, P], dt.float16, tag="iota")
            nc.sync.dma_start(iota_t[:], iota_d[:])
            w_t = cpool.tile([D, D], dt.float32, tag="w")
            nc.sync.dma_start(w_t[:], w_d[:])
            bvec_t = cpool.tile([P, 1], dt.float32, tag="bvec")
            nc.sync.dma_start(bvec_t[:], bvec_d[:])
            gnw_t = cpool.tile([P, 1], dt.float32, tag="gnw")
            nc.sync.dma_start(gnw_t[:], gnw_d[:])
            gnb_t = cpool.tile([P, 1], dt.float32, tag="gnb")
            nc.sync.dma_start(gnb_t[:], gnb_d[:])
            gms_t = cpool.tile([P, 1], dt.float32, tag="gms")
            nc.sync.dma_start(gms_t[:], gms_d[:])
            ginv_t = cpool.tile([P, GPC], dt.float32, tag="ginv")
            nc.sync.dma_start(ginv_t[:], ginv_d[:])
            ghole_t = cpool.tile([P, GPC], dt.float32, tag="ghole")
            nc.sync.dma_start(ghole_t[:], ghole_d[:])
            drel_t = mpool.tile([P, plan.VTOT], dt.float32, tag="drel")
            nc.sync.dma_start(drel_t[:], drel_d[:])
            attr_t = mpool.tile([P, plan.VTOT], dt.float32, tag="attr")
            nc.sync.dma_start(attr_t[:], attr_d[:])

            eps_t = cpool.tile([P, 1], dt.float32, tag="eps")
            nc.vector.memset(eps_t[:], EPS)
            # relu(b) for hole correction
            rb_t = cpool.tile([P, 1], dt.float32, tag="rb")
            nc.scalar.activation(rb_t[:], bvec_t[:], AF.Relu)
            rb2_t = cpool.tile([P, 1], dt.float32, tag="rb2")
            nc.vector.tensor_tensor(out=rb2_t[:], in0=rb_t[:], in1=rb_t[:],
                                    op=OP.mult)

            h_t = bigpool.tile([P, HTOT], dt.float32, tag="h")   # xagg then h
            scr_t = bigpool.tile([P, 512], dt.float32, tag="scr")

            vctr = 0
            seg_no = 0
            ST = SEG_T[0]
            for g in range(GPC):
                banks = [apool.tile([P, 512], dt.float32, tag="agg",
                                    name=f"aggb{g}_{i}")
                         for i in range((NB + 3) // 4)]
                # gather all 4 chunk segments of this graph up front, in two
                # halves per segment so buffer slots free mid-graph and the
                # next graph's gathers overlap this graph's compute.
                STH = (ST + 1) // 2
                halves = []                           # (c, half) -> (gb, t0, nt)
                for c in range(NCHUNK):
                    nvalid = int(NB * plan.cap[c])
                    for h in range(2):
                        t0 = h * STH
                        nt = min(STH, ST - t0)
                        if nt <= 0:
                            halves.append(None)
                            continue
                        v = min(max(nvalid - t0 * P, 0), nt * P)
                        icol = (g * NCHUNK + c) * ST * 8 + t0 * 8
                        idx_t = mpool.tile([P, nt * 8], dt.int16, tag="idxg",
                                           name=f"idxg{g}_{c}_{h}", bufs=12)
                        nc.sync.dma_start(idx_t[:],
                                          idx_d[:, icol:icol + nt * 8])
                        gb = gpool.tile([P, nt * P], dt.float16, tag="gb",
                                        name=f"gb{g}_{c}_{h}")
                        if seg_no < 12:
                            nc.vector.memset(gb[:], 0)  # no NaNs under pads
                        if v > 0:
                            nc.gpsimd.dma_gather(
                                out_ap=gb[:].rearrange("p (k e) -> p k e", e=P),
                                in_ap=x16_d[:, (c * D):(c + 1) * D],
                                idxs_ap=idx_t[:],
                                num_idxs=nt * P,
                                num_idxs_reg=v,
                                elem_size=D,
                                elem_step=4 * D,
                            )
                        seg_no += 1
                        halves.append((gb, t0, nt))

                def gb_slice(c, t):
                    h = 0 if t < STH else 1
                    ent = halves[c * 2 + h]
                    gb, t0, _nt = ent
                    tt = t - t0
                    return gb[:, tt * P:(tt + 1) * P]
                for b in range(NB):
                    bank, sub = divmod(b, 4)
                    for c in range(NCHUNK):
                        lo, hi = plan.win[(c, b)]
                        for t in range(lo, hi):
                            _g, _c, _b, _t, first, last = plan.visits[vctr]
                            assert (_g, _c, _b, _t) == (g, c, b, t)
                            s_t = spool.tile([P, P], dt.float16, tag="s")
                            nc.vector.tensor_scalar(
                                out=s_t[:], in0=iota_t[:],
                                scalar1=drel_t[:, vctr:vctr + 1],
                                scalar2=attr_t[:, vctr:vctr + 1],
                                op0=OP.is_equal, op1=OP.mult)
                            nc.tensor.matmul(
                                out=banks[bank][:, sub * P:(sub + 1) * P],
                                lhsT=gb_slice(c, t),
                                rhs=s_t[:],
                                start=first, stop=last)
                            vctr += 1
                # drain graph g: psum agg -> h_t (xagg region), then W matmul
                gbase = g * L
                for bank in range((NB + 3) // 4):
                    wdt = min(512, (NB - bank * 4) * P)
                    nc.scalar.copy(
                        out=h_t[:, gbase + bank * 512: gbase + bank * 512 + wdt],
                        in_=banks[bank][:, :wdt])
                # W matmul + relu+bias, 512 cols at a time, overwrite in place
                nsl = 0
                sums1 = []
                while nsl < L:
                    wdt = min(512, L - nsl)
                    wps = wpool.tile([P, 512], dt.float32, tag="wps")
                    nc.tensor.matmul(
                        out=wps[:, :wdt],
                        lhsT=w_t[:],
                        rhs=h_t[:, gbase + nsl: gbase + nsl + wdt],
                        start=True, stop=True)
                    s1 = scpool.tile([P, 1], dt.float32, tag=f"s1_{nsl}")
                    nc.scalar.activation(
                        out=h_t[:, gbase + nsl: gbase + nsl + wdt],
                        in_=wps[:, :wdt], func=AF.Relu,
                        bias=bvec_t[:, 0:1], scale=1.0,
                        accum_out=s1[:, 0:1])
                    sums1.append(s1)
                    nsl += wdt
                # second moment
                sums2 = []
                nsl = 0
                while nsl < L:
                    wdt = min(512, L - nsl)
                    s2 = scpool.tile([P, 1], dt.float32, tag=f"s2_{nsl}")
                    nc.scalar.activation(
                        out=scr_t[:, :wdt],
                        in_=h_t[:, gbase + nsl: gbase + nsl + wdt],
                        func=AF.Square, bias=0.0, scale=1.0,
                        accum_out=s2[:, 0:1])
                    sums2.append(s2)
                    nsl += wdt
                # reduce partials
                sum1 = scpool.tile([P, 1], dt.float32, tag="sum1")
                nc.vector.tensor_scalar(out=sum1[:], in0=sums1[0][:],
                                        scalar1=0.0, scalar2=None, op0=OP.add)
                for s1 in sums1[1:]:
                    nc.vector.tensor_tensor(out=sum1[:], in0=sum1[:],
                                            in1=s1[:], op=OP.add)
                sum2 = scpool.tile([P, 1], dt.float32, tag="sum2")
                nc.vector.tensor_scalar(out=sum2[:], in0=sums2[0][:],
                                        scalar1=0.0, scalar2=None, op0=OP.add)
                for s2 in sums2[1:]:
                    nc.vector.tensor_tensor(out=sum2[:], in0=sum2[:],
                                            in1=s2[:], op=OP.add)

                # hole corrections: sum1 -= holes*relu(b); sum2 -= holes*relu(b)^2
                tmp = scpool.tile([P, 1], dt.float32, tag="tmp")
                nc.vector.tensor_tensor(out=tmp[:], in0=ghole_t[:, g:g + 1],
                                        in1=rb_t[:], op=OP.mult)
                nc.vector.tensor_tensor(out=sum1[:], in0=sum1[:], in1=tmp[:],
                                        op=OP.subtract)
                nc.vector.tensor_tensor(out=tmp[:], in0=ghole_t[:, g:g + 1],
                                        in1=rb2_t[:], op=OP.mult)
                nc.vector.tensor_tensor(out=sum2[:], in0=sum2[:], in1=tmp[:],
                                        op=OP.subtract)

                # mean = sum1*inv; m2 = mean*gms
                mean = scpool.tile([P, 1], dt.float32, tag="mean")
                nc.vector.tensor_tensor(out=mean[:], in0=sum1[:],
                                        in1=ginv_t[:, g:g + 1], op=OP.mult)
                m2 = scpool.tile([P, 1], dt.float32, tag="m2")
                nc.vector.tensor_tensor(out=m2[:], in0=mean[:], in1=gms_t[:],
                                        op=OP.mult)
                # var = sum2*inv - m2*(2*mean - m2)
                var = scpool.tile([P, 1], dt.float32, tag="var")
                nc.vector.tensor_tensor(out=var[:], in0=sum2[:],
                                        in1=ginv_t[:, g:g + 1], op=OP.mult)
                t2m = scpool.tile([P, 1], dt.float32, tag="t2m")
                nc.vector.tensor_tensor(out=t2m[:], in0=mean[:], in1=mean[:],
                                        op=OP.add)
                nc.vector.tensor_tensor(out=t2m[:], in0=t2m[:], in1=m2[:],
                                        op=OP.subtract)
                nc.vector.tensor_tensor(out=t2m[:], in0=t2m[:], in1=m2[:],
                                        op=OP.mult)
                nc.vector.tensor_tensor(out=var[:], in0=var[:], in1=t2m[:],
                                        op=OP.subtract)
                # rstd = 1/sqrt(var + eps)
                std = scpool.tile([P, 1], dt.float32, tag="std")
                nc.scalar.activation(std[:], var[:], AF.Sqrt,
                                     bias=eps_t[:, 0:1], scale=1.0)
                rstd = scpool.tile([P, 1], dt.float32, tag="rstd")
                nc.vector.reciprocal(rstd[:], std[:])
                # scale = gnw*rstd; shift = m2*scale - gnb
                scale = scpool.tile([P, 1], dt.float32, tag="scale")
                nc.vector.tensor_tensor(out=scale[:], in0=gnw_t[:], in1=rstd[:],
                                        op=OP.mult)
                shift = scpool.tile([P, 1], dt.float32, tag="shift")
                nc.vector.tensor_tensor(out=shift[:], in0=m2[:], in1=scale[:],
                                        op=OP.mult)
                nc.vector.tensor_tensor(out=shift[:], in0=shift[:], in1=gnb_t[:],
                                        op=OP.subtract)
                # out = h*scale - shift
                nc.vector.tensor_scalar(
                    out=h_t[:, gbase: gbase + L],
                    in0=h_t[:, gbase: gbase + L],
                    scalar1=scale[:, 0:1], scalar2=shift[:, 0:1],
                    op0=OP.mult, op1=OP.subtract)
                nc.sync.dma_start(out_d[:, gbase: gbase + L],
                                  h_t[:, gbase: gbase + L])

            assert vctr == plan.VTOT
    nc.compile()
    return nc


# --------------------------------------------------------------------------
# Entry point
# --------------------------------------------------------------------------

def _ensure_axon_hooks():
    """Provide antenv.axon_hooks if the container image lacks it.

    bass_utils imports it unconditionally when BASS_TRACE is set under axon;
    we supply the same ctypes-based NTFF hook the axon boot would install.
    """
    import types
    import contextlib
    import ctypes
    try:
        import antenv.axon_hooks  # noqa: F401
        return
    except Exception:
        pass
    try:
        import antenv
    except Exception:
        return
    state = {"hook": None, "made": False}

    def _make_hook():
        so_path = "/opt/axon/libaxon_pjrt.so"
        try:
            lib = ctypes.CDLL(so_path)
        except OSError:
            return None
        if not hasattr(lib, "axon_start_nrt_profile"):
            return None
        lib.axon_start_nrt_profile.argtypes = [
            ctypes.POINTER(ctypes.c_int64), ctypes.c_size_t]
        lib.axon_start_nrt_profile.restype = ctypes.c_int64
        lib.axon_stop_nrt_profile.argtypes = [ctypes.c_char_p]
        lib.axon_stop_nrt_profile.restype = ctypes.c_int64

        @contextlib.contextmanager
        def _hook(output_dir, device_ids):
            import jax
            jax.devices()
            if device_ids:
                ids = (ctypes.c_int64 * len(device_ids))(*device_ids)
                rc = lib.axon_start_nrt_profile(ids, len(device_ids))
            else:
                rc = lib.axon_start_nrt_profile(None, 0)
            if rc != 0:
                raise RuntimeError(f"axon_start_nrt_profile rc={rc}")
            try:
                yield
            finally:
                n = lib.axon_stop_nrt_profile(str(output_dir).encode())
                print(f"ntff profile: {n} file(s) -> {output_dir}")
        return _hook

    mod = types.ModuleType("antenv.axon_hooks")

    def set_axon_ntff_profile_hook(h):
        state["hook"] = h
        state["made"] = True

    def get_axon_ntff_profile_hook():
        if not state["made"]:
            state["hook"] = _make_hook()
            state["made"] = True
        return state["hook"]

    mod.set_axon_ntff_profile_hook = set_axon_ntff_profile_hook
    mod.get_axon_ntff_profile_hook = get_axon_ntff_profile_hook
    sys.modules["antenv.axon_hooks"] = mod
    antenv.axon_hooks = mod


def kernel(**inputs):
    global LAST_RESULT
    _ensure_axon_hooks()
    from concourse.bass_utils import run_bass_kernel_spmd

    x = np.asarray(inputs["x"], np.float32)
    edge_index = np.asarray(inputs["edge_index"]).astype(np.int64)
    edge_attr = np.asarray(inputs["edge_attr"], np.float32)
    batch_index = np.asarray(inputs["batch_index"]).astype(np.int64)
    W = np.asarray(inputs["W"], np.float32)
    b = np.asarray(inputs["b"], np.float32)
    gnw = np.asarray(inputs["gn_weight"], np.float32)
    gnb = np.asarray(inputs["gn_bias"], np.float32)
    gms = np.asarray(inputs["gn_mean_scale"], np.float32)

    N = x.shape[0]
    plan = _build_plan(x, edge_index, edge_attr, batch_index)
    nc = _build_program(plan)

    # shared inputs
    x16 = x.astype(np.float16)
    # chunk-major row layout: row r of x16_d = [x[4r], x[4r+1], x[4r+2], x[4r+3]]
    # -> column block c holds parity-c rows; matches in_ap slicing.
    assert N % 4 == 0
    x16_tab = x16.reshape(N // 4, 4 * D)
    iota_np = np.tile(np.arange(P, dtype=np.float16)[None, :], (P, 1))

    in_maps = []
    for core in range(N_CORES):
        idx16, drel_v, attr_v = _build_core_arrays(plan, core)
        cnt = plan.n_g[core * plan.GPC:(core + 1) * plan.GPC].astype(np.float32)
        inv = 1.0 / np.maximum(cnt, 1.0)
        holes = plan.L - cnt
        in_maps.append(dict(
            x16=x16_tab, idx=idx16, drel=drel_v, attr=attr_v, iota=iota_np,
            w=W, bvec=b.reshape(P, 1), gnw=gnw.reshape(P, 1),
            gnb=gnb.reshape(P, 1), gms=gms.reshape(P, 1),
            ginv=np.tile(inv[None, :], (P, 1)).astype(np.float32),
            ghole=np.tile(holes[None, :], (P, 1)).astype(np.float32),
        ))

    res = run_bass_kernel_spmd(nc, in_maps, core_ids=list(range(N_CORES)))
    LAST_RESULT = res

    # unshard: out[core] is [128 d, GPC*L slots]
    out = np.empty((N, D), np.float32)
    for core in range(N_CORES):
        oh = res.results[core]["out"]                   # [D, GPC*L]
        for g in range(plan.GPC):
            gg = core * plan.GPC + g
            nodes = np.arange(plan.gstart[gg], plan.gend[gg])
            slots = plan.slot_of_node[nodes] + g * plan.L
            out[nodes] = oh[:, slots].T
    return out
